# revision 3
# baseline (speedup 1.0000x reference)
"""Trainium2 Bass kernel for CepstralBlock: causal depthwise conv along D
(K=4, per-channel weights) followed by a 128x128 Linear.

Math trick (v2): fold the 4 taps PAIRWISE into the moving operand.
With V0 = diag(w0) W and V2 = diag(w2) W:

    out_d = V0^T (x_d + r1*x_{d-1}) + V2^T (x_{d-2} + r3*x_{d-3})
          = V0^T v_d + V2^T u_{d-2}
    r1 = w1/w0,  r3 = w3/w2   (per-channel, computed on host, guarded)

v and u are one fused DVE scalar_tensor_tensor op each (out = (in0*scalar)
+ in1, scalar per-partition), so the PE does 2 matmuls per depth slice
instead of 4.  Exact algebra: V0^T r1 v-part = w0*(w1/w0) x W = w1 x W.
When |w0| (or |w2|) is ~0 the guarded denominator introduces error
<= 1e-12*|xW| - negligible.

PSUM evacuation (+bias, f32->bf16) runs on the otherwise-idle Scalar
engine (activation Identity with per-partition bias), freeing the DVE.

Layout: host pre-transposes x to channel-major [C, B, D, S]; C=128 on
the SBUF partition axis (matmul contraction), no on-device transposes.
Sharding: data-parallel over H (64 -> 8 per core), 8 cores, no
collectives.  bf16 in/out, f32 PSUM accumulate.

Engine budget per core (target ~47us DMA-bound):
  DMA  ~47us (8.4 MB in + 8.4 MB out @ ~358 GB/s)  <- wall
  PE   ~27us (120 matmuls FD=512/1024)
  DVE  ~32us (stt passes for v, u)
  ACT  ~20-30us (PSUM evac + bias)
"""

import sys
import types

sys.path.insert(0, "/opt/trn_rl_repo")

import numpy as np
import ml_dtypes

# Problem shapes (hardcoded; kernel.py must be self-contained).
B = 4
D = 16
H = 64
WD = 64
C = 128
KS = 4
NCORES = 8
HSH = H // NCORES          # 8 H-rows per core
S = HSH * WD               # 512 spatial positions per (b, d) slice

# Set by test.py to run with NTFF profiling and stash exec time here.
PROFILE = False
NPROF = 4          # traced runs when PROFILE; min exec_time_ns is reported
LAST_EXEC_NS = None
LAST_EXEC_ALL = None
LAST_RESULT = None

_graph_cache = {}


def _install_ntff_hook():
    """Provide antenv.axon_hooks + register the NTFF profile hook if the
    image's antenv package lacks it (needed for trace=True under axon)."""
    try:
        from antenv import axon_hooks  # noqa: F401
        return
    except ImportError:
        pass
    try:
        import antenv
        from trn_agent_boot.trn_boot import _ntff_profile_via_ctypes
    except ImportError:
        return
    mod = types.ModuleType("antenv.axon_hooks")
    mod._hook = None

    def set_axon_ntff_profile_hook(h):
        mod._hook = h

    def get_axon_ntff_profile_hook():
        return mod._hook

    mod.set_axon_ntff_profile_hook = set_axon_ntff_profile_hook
    mod.get_axon_ntff_profile_hook = get_axon_ntff_profile_hook
    sys.modules["antenv.axon_hooks"] = mod
    antenv.axon_hooks = mod
    mod.set_axon_ntff_profile_hook(
        _ntff_profile_via_ctypes("/opt/axon/libaxon_pjrt.so")
    )


def _build_graph():
    import concourse.mybir as mybir
    from concourse import bacc
    from concourse.tile import TileContext

    nc = bacc.Bacc("TRN2", target_bir_lowering=False, debug=False,
                   num_devices=NCORES)
    xt = nc.declare_dram_parameter("xt", [C, B, D, S], mybir.dt.bfloat16,
                                   isOutput=False)
    # wk = [V0 | V2], V0 = diag(w0)W, V2 = diag(w2)W
    wk = nc.declare_dram_parameter("wk", [C, 2 * C], mybir.dt.bfloat16,
                                   isOutput=False)
    # misc columns: 0 = bias, 1 = r1 = w1/w0, 2 = r3 = w3/w2
    misc = nc.declare_dram_parameter("misc", [C, 3], mybir.dt.float32,
                                     isOutput=False)
    out = nc.declare_dram_parameter("out", [C, B, D, S], mybir.dt.bfloat16,
                                    isOutput=True)

    MULT = mybir.AluOpType.mult
    ADD = mybir.AluOpType.add
    IDENT = mybir.ActivationFunctionType.Identity

    with TileContext(nc) as tc:
        with (
            tc.tile_pool(name="consts", bufs=1) as cpool,
            tc.tile_pool(name="xin", bufs=6) as xpool,
            tc.tile_pool(name="vconv", bufs=2) as vpool,
            tc.tile_pool(name="uconv", bufs=2) as upool,
            tc.tile_pool(name="ostage", bufs=4) as opool,
            tc.tile_pool(name="ps", bufs=4, space="PSUM") as pspool,
        ):
            wk_sb = cpool.tile([C, 2 * C], mybir.dt.bfloat16)
            nc.sync.dma_start(out=wk_sb[:], in_=wk[:])
            # misc rides the ACT HWDGE ring: its [128 x 12B] descriptor
            # swarm must not delay the x loads on the SP ring.
            m_sb = cpool.tile([C, 3], mybir.dt.float32)
            nc.scalar.dma_start(out=m_sb[:], in_=misc[:])
            b_sb = m_sb[:, 0:1]
            r1_sb = m_sb[:, 1:2]
            r3_sb = m_sb[:, 2:3]

            # Keep the PE busy while the first x chunks stream in, so the
            # HAM clock gate flips to 2.4 GHz before the real matmuls start.
            warm_src = cpool.tile([C, S], mybir.dt.bfloat16)
            nc.vector.memset(warm_src[:], 0.0)
            for i in range(4):
                wt = pspool.tile([C, 2 * S], mybir.dt.float32, tag="ps",
                                 name=f"warm_{i}")
                nc.tensor.matmul(wt[:, 0:S], warm_src[:, 0:C], warm_src[:],
                                 start=True, stop=True)

            for b in range(B):
                load_chunks = [(0, 2), (2, 2), (4, 4), (8, 8)] if b == 0 \
                    else [(0, 8), (8, 8)]

                xh = []
                for st, ln in load_chunks:
                    t = xpool.tile([C, ln * S], mybir.dt.bfloat16, tag="xh",
                                   name=f"xh_{b}_{st}")
                    nc.sync.dma_start(
                        out=t[:],
                        in_=xt[:, b, st:st + ln].rearrange("c d s -> c (d s)"),
                    )
                    xh.append((st, ln, t))

                def x_span(t0, t1):
                    """AP covering x_{t0..t1-1}; must lie in one chunk."""
                    for st, ln, t in xh:
                        if st <= t0 and t1 <= st + ln:
                            return t[:, (t0 - st) * S:(t1 - st) * S]
                    raise AssertionError((t0, t1))

                # v_t = x_t + r1*x_{t-1} for t in [1,16); slot t*S.
                # u_t = x_t + r3*x_{t-1} for t in [1,14); slot t*S.
                # (v_0 = u_0 = x_0 are aliased straight from the x tile.)
                vt = vpool.tile([C, D * S], mybir.dt.bfloat16, tag="vt",
                                name=f"vt_{b}")
                ut = upool.tile([C, (D - 2) * S], mybir.dt.bfloat16, tag="ut",
                                name=f"ut_{b}")

                def emit_stt(dst, r_sb, hi):
                    # split [1, hi) at chunk starts and starts+1 so each op
                    # reads x_{t-1} and x_t from fixed tiles
                    cuts = {1, hi}
                    for st, _ln in load_chunks:
                        for c in (st, st + 1):
                            if 1 <= c <= hi:
                                cuts.add(c)
                    cuts = sorted(cuts)
                    for t0, t1 in zip(cuts[:-1], cuts[1:]):
                        nc.vector.scalar_tensor_tensor(
                            out=dst[:, t0 * S:t1 * S],
                            in0=x_span(t0 - 1, t1 - 1),
                            scalar=r_sb,
                            in1=x_span(t0, t1),
                            op0=MULT,
                            op1=ADD,
                        )

                emit_stt(vt, r1_sb, D)
                emit_stt(ut, r3_sb, D - 2)

                def v_ap(d0, d1):
                    if d0 == 0:
                        assert d1 == 1
                        return x_span(0, 1)
                    return vt[:, d0 * S:d1 * S]

                def u_ap(t0, t1):
                    if t0 == 0:
                        assert t1 == 1
                        return x_span(0, 1)
                    return ut[:, t0 * S:t1 * S]

                ostages = []
                for h in range(2):
                    ot = opool.tile([C, 8 * S], mybir.dt.bfloat16, tag="ot",
                                    name=f"ot_{b}_{h}")
                    ostages.append(ot)

                # blocks of 2 depth slices -> one [C, 2S] PSUM tile
                # (2 banks; one matmul may only write one bank, so each
                # depth gets its own FD=512 matmuls).  Order V0,V0,V2,V2
                # so the stationary weights reload only twice per block.
                for j in range(D // 2):
                    ps = pspool.tile([C, 2 * S], mybir.dt.float32, tag="ps",
                                     name=f"ps_{b}_{j}")
                    for i, d in ((0, 2 * j), (1, 2 * j + 1)):
                        nc.tensor.matmul(ps[:, i * S:(i + 1) * S],
                                         wk_sb[:, 0:C], v_ap(d, d + 1),
                                         start=True, stop=(d - 2 < 0))
                    for i, d in ((0, 2 * j), (1, 2 * j + 1)):
                        tu = d - 2          # u index for this depth
                        if tu >= 0:
                            nc.tensor.matmul(ps[:, i * S:(i + 1) * S],
                                             wk_sb[:, C:2 * C],
                                             u_ap(tu, tu + 1),
                                             start=False, stop=True)
                    h = j // 4
                    nc.scalar.activation(
                        out=ostages[h][:, (2 * j - 8 * h) * S:
                                       (2 * j - 8 * h + 2) * S],
                        in_=ps[:],
                        func=IDENT,
                        bias=b_sb,
                        scale=1.0,
                    )
                    if j % 4 == 3:
                        nc.scalar.dma_start(
                            out=out[:, b, 8 * h:8 * h + 8].rearrange(
                                "c d s -> c (d s)"),
                            in_=ostages[h][:],
                        )
    nc.compile()
    return nc


def _get_graph():
    if "nc" not in _graph_cache:
        _graph_cache["nc"] = _build_graph()
    return _graph_cache["nc"]


def kernel(x, kernel, W, b):
    global LAST_EXEC_NS, LAST_RESULT
    from concourse.bass_utils import run_bass_kernel_spmd

    nc = _get_graph()

    x = np.asarray(x, np.float32)
    kernel = np.asarray(kernel, np.float32)
    W = np.asarray(W, np.float32)
    b = np.asarray(b, np.float32)

    # Host precompute: pairwise tap folding.
    w_full = np.tile(kernel, (C // kernel.shape[0], 1))          # [C, KS]
    w0, w1, w2, w3 = (w_full[:, k] for k in range(KS))
    eps = np.float32(1e-12)
    g0 = np.where(np.abs(w0) < eps, np.copysign(eps, w0), w0)
    g2 = np.where(np.abs(w2) < eps, np.copysign(eps, w2), w2)
    wk_cat = np.concatenate(
        [g0[:, None] * W, g2[:, None] * W], axis=1        # [C, 2C]
    ).astype(ml_dtypes.bfloat16)
    misc = np.stack(
        [b, w1 / g0, w3 / g2], axis=1                      # [C, 3]
    ).astype(np.float32)

    # Channel-major transpose + H-shard + bf16.
    xbf = x.astype(ml_dtypes.bfloat16)
    xtr = np.transpose(xbf, (4, 0, 1, 2, 3))                     # [C,B,D,H,W]
    in_maps = []
    for i in range(NCORES):
        shard = np.ascontiguousarray(
            xtr[:, :, :, i * HSH:(i + 1) * HSH, :]
        ).reshape(C, B, D, S)
        in_maps.append({"xt": shard, "wk": wk_cat, "misc": misc})

    global LAST_EXEC_ALL
    core_ids = list(range(NCORES))
    res = None
    if PROFILE:
        _install_ntff_hook()
        try:
            # Warm run first: the NEFF compile on a cold cache must not
            # happen inside the NTFF capture window.
            run_bass_kernel_spmd(nc, in_maps, core_ids=core_ids)
            times = []
            for _ in range(max(1, NPROF)):
                res = run_bass_kernel_spmd(nc, in_maps, core_ids=core_ids,
                                           trace=True)
                times.append(res.exec_time_ns)
            LAST_EXEC_ALL = times
        except Exception as e:
            print(f"profile run failed ({type(e).__name__}: {e}); "
                  "falling back to non-traced run", file=sys.stderr)
            res = None
    if res is None:
        res = run_bass_kernel_spmd(nc, in_maps, core_ids=core_ids)
        LAST_EXEC_NS = res.exec_time_ns
    else:
        LAST_EXEC_NS = min(t for t in LAST_EXEC_ALL if t is not None)
    LAST_RESULT = res

    # Gather: shard_i[o, b, d, h*WD + w] -> full[b, d, i*HSH + h, w, o]
    o = np.stack([np.asarray(res.results[i]["out"]) for i in range(NCORES)],
                 axis=0).astype(np.float32)
    o = o.reshape(NCORES, C, B, D, HSH, WD)
    o = np.transpose(o, (2, 3, 0, 4, 5, 1)).reshape(B, D, H, WD, C)
    return np.ascontiguousarray(o)


# revision 7
# speedup vs baseline: 1.4188x; 1.4188x over previous
"""Trainium2 Bass kernel for CepstralBlock: causal depthwise conv along D
(K=4, per-channel weights) followed by a 128x128 Linear.

Math trick (v3): taps 0,1 stay as PSUM-accumulated matmuls; taps 2,3 are
folded PAIRWISE into the moving operand.  With Wk = diag(wk) W:

    out_d = W0^T x_d + W1^T x_{d-1} + W2^T (x_{d-2} + r3*x_{d-3})
          = W0^T x_d + W1^T x_{d-1} + W2^T u_{d-2}
    r3 = w3/w2   (per-channel, host-computed, guarded denominator)

u costs two cheap DVE passes (tensor_scalar_mul runs in 4x mode,
tensor_add in 2x mode -- the fused scalar_tensor_tensor op only has a 1x
uop and measured 2x slower than this pair).  The PE does 3 matmuls per
depth slice instead of 4 (measured PE-bound at 4).  Exact algebra:
W2^T r3 u-part = w2*(w3/w2) x W = w3 x W.  When |w2| ~ 0 the guarded
denominator introduces error <= 1e-12*|xW| -- negligible.

PSUM evacuation (+bias, f32->bf16) runs on the otherwise-idle Scalar
engine (activation Identity with per-partition bias), freeing the DVE.

Layout: host pre-transposes x to channel-major [C, B, D, S]; C=128 on
the SBUF partition axis (matmul contraction), no on-device transposes.
Sharding: data-parallel over H (64 -> 8 per core), 8 cores, no
collectives.  bf16 in/out, f32 PSUM accumulate.

Engine budget per core (target ~48us DMA-bound work window):
  DMA  ~47.6us (8.4 MB in + 8.4 MB out @ ~358 GB/s)  <- wall
  PE   ~41us (180 matmuls FD=512 + cold-clock ramp)
  ACT  ~36us (PSUM evac + bias)
  DVE  ~22us (s = r3*x, u = s_shift + x)
"""

import sys
import types

sys.path.insert(0, "/opt/trn_rl_repo")

import numpy as np
import ml_dtypes

# Problem shapes (hardcoded; kernel.py must be self-contained).
B = 4
D = 16
H = 64
WD = 64
C = 128
KS = 4
NCORES = 8
HSH = H // NCORES          # 8 H-rows per core
S = HSH * WD               # 512 spatial positions per (b, d) slice

# Set by test.py to run with NTFF profiling and stash exec time here.
PROFILE = False
NPROF = 4          # traced runs when PROFILE; min exec_time_ns is reported
LAST_EXEC_NS = None
LAST_EXEC_ALL = None
LAST_RESULT = None

_graph_cache = {}


def _install_ntff_hook():
    """Provide antenv.axon_hooks + register the NTFF profile hook if the
    image's antenv package lacks it (needed for trace=True under axon)."""
    try:
        from antenv import axon_hooks  # noqa: F401
        return
    except ImportError:
        pass
    try:
        import antenv
        from trn_agent_boot.trn_boot import _ntff_profile_via_ctypes
    except ImportError:
        return
    mod = types.ModuleType("antenv.axon_hooks")
    mod._hook = None

    def set_axon_ntff_profile_hook(h):
        mod._hook = h

    def get_axon_ntff_profile_hook():
        return mod._hook

    mod.set_axon_ntff_profile_hook = set_axon_ntff_profile_hook
    mod.get_axon_ntff_profile_hook = get_axon_ntff_profile_hook
    sys.modules["antenv.axon_hooks"] = mod
    antenv.axon_hooks = mod
    mod.set_axon_ntff_profile_hook(
        _ntff_profile_via_ctypes("/opt/axon/libaxon_pjrt.so")
    )


def _build_graph():
    import concourse.mybir as mybir
    from concourse import bacc
    from concourse.tile import TileContext

    nc = bacc.Bacc("TRN2", target_bir_lowering=False, debug=False,
                   num_devices=NCORES)
    xt = nc.declare_dram_parameter("xt", [C, B, D, S], mybir.dt.bfloat16,
                                   isOutput=False)
    # wk = [W0 | W1 | W2], Wk = diag(wk) W  (W2 with guarded w2)
    wk = nc.declare_dram_parameter("wk", [C, 3 * C], mybir.dt.bfloat16,
                                   isOutput=False)
    # misc columns: 0 = bias, 1 = r3 = w3/w2
    misc = nc.declare_dram_parameter("misc", [C, 2], mybir.dt.float32,
                                     isOutput=False)
    out = nc.declare_dram_parameter("out", [C, B, D, S], mybir.dt.bfloat16,
                                    isOutput=True)

    IDENT = mybir.ActivationFunctionType.Identity
    NU = D - 2                  # u_t defined for t in [0, NU); u_0 = x_0

    with TileContext(nc) as tc:
        with (
            tc.tile_pool(name="consts", bufs=1) as cpool,
            tc.tile_pool(name="xin", bufs=9) as xpool,
            tc.tile_pool(name="sconv", bufs=2) as spool,
            tc.tile_pool(name="uconv", bufs=2) as upool,
            tc.tile_pool(name="ostage", bufs=6) as opool,
            tc.tile_pool(name="ps", bufs=4, space="PSUM") as pspool,
        ):
            # Both const loads ride the ACT HWDGE ring so the x loads are
            # first in the SP ring's FIFO.
            wk_sb = cpool.tile([C, 3 * C], mybir.dt.bfloat16)
            nc.scalar.dma_start(out=wk_sb[:], in_=wk[:])
            # misc rides the ACT HWDGE ring: its [128 x 8B] descriptor
            # swarm must not delay the x loads on the SP ring.
            m_sb = cpool.tile([C, 2], mybir.dt.float32)
            nc.scalar.dma_start(out=m_sb[:], in_=misc[:])
            b_sb = m_sb[:, 0:1]
            r3_sb = m_sb[:, 1:2]

            # Keep the PE busy while the first x chunks stream in, so the
            # HAM clock gate flips to 2.4 GHz before the real matmuls start.
            warm_src = cpool.tile([C, S], mybir.dt.bfloat16)
            nc.vector.memset(warm_src[:], 0.0)
            for i in range(4):
                wt = pspool.tile([C, 2 * S], mybir.dt.float32, tag="ps",
                                 name=f"warm_{i}")
                nc.tensor.matmul(wt[:, 0:S], warm_src[:, 0:C], warm_src[:],
                                 start=True, stop=True)

            for b in range(B):
                load_chunks = [(0, 2), (2, 2), (4, 4), (8, 8)] if b == 0 \
                    else [(0, 8), (8, 8)]

                xh = []
                for st, ln in load_chunks:
                    t = xpool.tile([C, ln * S], mybir.dt.bfloat16, tag="xh",
                                   name=f"xh_{b}_{st}")
                    nc.sync.dma_start(
                        out=t[:],
                        in_=xt[:, b, st:st + ln].rearrange("c d s -> c (d s)"),
                    )
                    xh.append((st, ln, t))

                def x_span(t0, t1):
                    """AP covering x_{t0..t1-1}; must lie in one chunk."""
                    for st, ln, t in xh:
                        if st <= t0 and t1 <= st + ln:
                            return t[:, (t0 - st) * S:(t1 - st) * S]
                    raise AssertionError((t0, t1))

                # s_t = r3*x_t   for t in [0, NU-1)   (per load chunk)
                # u_t = s_{t-1} + x_t  for t in [1, NU); slot t*S in ut.
                # (u_0 = x_0 is aliased straight from the x tile.)
                st_tile = spool.tile([C, (NU - 1) * S], mybir.dt.bfloat16,
                                     tag="st", name=f"st_{b}")
                ut = upool.tile([C, NU * S], mybir.dt.bfloat16, tag="ut",
                                name=f"ut_{b}")

                for st, ln in load_chunks:
                    t0, t1 = st, min(NU - 1, st + ln)
                    if t0 < t1:
                        nc.vector.tensor_scalar_mul(
                            st_tile[:, t0 * S:t1 * S], x_span(t0, t1), r3_sb)

                # split [1, NU) at chunk starts so each tensor_add reads
                # x_t from a fixed tile (s is monolithic per b)
                cuts = sorted({1, NU} | {st for st, _ in load_chunks
                                         if 1 <= st <= NU})
                for t0, t1 in zip(cuts[:-1], cuts[1:]):
                    nc.vector.tensor_add(
                        ut[:, t0 * S:t1 * S],
                        st_tile[:, (t0 - 1) * S:(t1 - 1) * S],
                        x_span(t0, t1))

                def u_ap(tu):
                    if tu == 0:
                        return x_span(0, 1)
                    return ut[:, tu * S:(tu + 1) * S]

                # one staging tile per 4 depths -> 0.52 MB stores: starts
                # earlier and shortens the last evac->store drain tail
                ostages = []
                for h in range(4):
                    ot = opool.tile([C, 4 * S], mybir.dt.bfloat16, tag="ot",
                                    name=f"ot_{b}_{h}")
                    ostages.append(ot)

                # blocks of 2 depth slices -> one [C, 2S] PSUM tile
                # (2 banks; one matmul may only write one bank, so each
                # depth gets its own FD=512 matmuls).  Order W0,W0,W1,W1,
                # W2,W2 so the stationary reloads 3x per block, hidden
                # under the FD=512 matmul stream.
                for j in range(D // 2):
                    ps = pspool.tile([C, 2 * S], mybir.dt.float32, tag="ps",
                                     name=f"ps_{b}_{j}")

                    def psl(i):
                        return ps[:, i * S:(i + 1) * S]

                    dpair = (2 * j, 2 * j + 1)
                    for i, d in enumerate(dpair):
                        nc.tensor.matmul(psl(i), wk_sb[:, 0:C],
                                         x_span(d, d + 1),
                                         start=True, stop=(d == 0))
                    for i, d in enumerate(dpair):
                        if d >= 1:
                            nc.tensor.matmul(psl(i), wk_sb[:, C:2 * C],
                                             x_span(d - 1, d),
                                             start=False, stop=(d == 1))
                    for i, d in enumerate(dpair):
                        if d >= 2:
                            nc.tensor.matmul(psl(i), wk_sb[:, 2 * C:3 * C],
                                             u_ap(d - 2),
                                             start=False, stop=True)
                    h = j // 2
                    nc.scalar.activation(
                        out=ostages[h][:, (2 * j - 4 * h) * S:
                                       (2 * j - 4 * h + 2) * S],
                        in_=ps[:],
                        func=IDENT,
                        bias=b_sb,
                        scale=1.0,
                    )
                    if j % 2 == 1:
                        nc.scalar.dma_start(
                            out=out[:, b, 4 * h:4 * h + 4].rearrange(
                                "c d s -> c (d s)"),
                            in_=ostages[h][:],
                        )
    nc.compile()
    return nc


def _get_graph():
    if "nc" not in _graph_cache:
        _graph_cache["nc"] = _build_graph()
    return _graph_cache["nc"]


def kernel(x, kernel, W, b):
    global LAST_EXEC_NS, LAST_RESULT
    from concourse.bass_utils import run_bass_kernel_spmd

    nc = _get_graph()

    x = np.asarray(x, np.float32)
    kernel = np.asarray(kernel, np.float32)
    W = np.asarray(W, np.float32)
    b = np.asarray(b, np.float32)

    # Host precompute: tap weights folded into the Linear; taps 2,3 share
    # W2 via the ratio r3 (exact: w2*(w3/w2) = w3).
    w_full = np.tile(kernel, (C // kernel.shape[0], 1))          # [C, KS]
    w0, w1, w2, w3 = (w_full[:, k] for k in range(KS))
    eps = np.float32(1e-12)
    g2 = np.where(np.abs(w2) < eps, np.copysign(eps, w2), w2)
    wk_cat = np.concatenate(
        [w0[:, None] * W, w1[:, None] * W, g2[:, None] * W], axis=1  # [C,3C]
    ).astype(ml_dtypes.bfloat16)
    misc = np.stack([b, w3 / g2], axis=1).astype(np.float32)     # [C, 2]

    # Channel-major transpose + H-shard + bf16.
    xbf = x.astype(ml_dtypes.bfloat16)
    xtr = np.transpose(xbf, (4, 0, 1, 2, 3))                     # [C,B,D,H,W]
    in_maps = []
    for i in range(NCORES):
        shard = np.ascontiguousarray(
            xtr[:, :, :, i * HSH:(i + 1) * HSH, :]
        ).reshape(C, B, D, S)
        in_maps.append({"xt": shard, "wk": wk_cat, "misc": misc})

    global LAST_EXEC_ALL
    core_ids = list(range(NCORES))
    res = None
    if PROFILE:
        _install_ntff_hook()
        try:
            # Warm run first: the NEFF compile on a cold cache must not
            # happen inside the NTFF capture window.
            run_bass_kernel_spmd(nc, in_maps, core_ids=core_ids)
            times = []
            for _ in range(max(1, NPROF)):
                res = run_bass_kernel_spmd(nc, in_maps, core_ids=core_ids,
                                           trace=True)
                times.append(res.exec_time_ns)
            LAST_EXEC_ALL = times
        except Exception as e:
            print(f"profile run failed ({type(e).__name__}: {e}); "
                  "falling back to non-traced run", file=sys.stderr)
            res = None
    if res is None:
        res = run_bass_kernel_spmd(nc, in_maps, core_ids=core_ids)
        LAST_EXEC_NS = res.exec_time_ns
    else:
        LAST_EXEC_NS = min(t for t in LAST_EXEC_ALL if t is not None)
    LAST_RESULT = res

    # Gather: shard_i[o, b, d, h*WD + w] -> full[b, d, i*HSH + h, w, o]
    o = np.stack([np.asarray(res.results[i]["out"]) for i in range(NCORES)],
                 axis=0).astype(np.float32)
    o = o.reshape(NCORES, C, B, D, HSH, WD)
    o = np.transpose(o, (2, 3, 0, 4, 5, 1)).reshape(B, D, H, WD, C)
    return np.ascontiguousarray(o)


# revision 9
# speedup vs baseline: 1.4787x; 1.0423x over previous
"""Trainium2 Bass kernel for CepstralBlock: causal depthwise conv along D
(K=4, per-channel weights) followed by a 128x128 Linear.

Math trick (v3): taps 0,1 stay as PSUM-accumulated matmuls; taps 2,3 are
folded PAIRWISE into the moving operand.  With Wk = diag(wk) W:

    out_d = W0^T x_d + W1^T x_{d-1} + W2^T (x_{d-2} + r3*x_{d-3})
          = W0^T x_d + W1^T x_{d-1} + W2^T u_{d-2}
    r3 = w3/w2   (per-channel, host-computed, guarded denominator)

u costs two cheap DVE passes (tensor_scalar_mul runs in 4x mode,
tensor_add in 2x mode -- the fused scalar_tensor_tensor op only has a 1x
uop and measured 2x slower than this pair).  The PE does 3 matmuls per
depth slice instead of 4 (measured PE-bound at 4).  Exact algebra:
W2^T r3 u-part = w2*(w3/w2) x W = w3 x W.  When |w2| ~ 0 the guarded
denominator introduces error <= 1e-12*|xW| -- negligible.

PSUM evacuation (+bias, f32->bf16) runs on the otherwise-idle Scalar
engine (activation Identity with per-partition bias), freeing the DVE.

Layout: host pre-transposes x to channel-major [C, B, D, S]; C=128 on
the SBUF partition axis (matmul contraction), no on-device transposes.
Sharding: data-parallel over H (64 -> 8 per core), 8 cores, no
collectives.  bf16 in/out, f32 PSUM accumulate.

Engine budget per core (target ~48us DMA-bound work window):
  DMA  ~47.6us (8.4 MB in + 8.4 MB out @ ~358 GB/s)  <- wall
  PE   ~41us (180 matmuls FD=512 + cold-clock ramp)
  ACT  ~36us (PSUM evac + bias)
  DVE  ~22us (s = r3*x, u = s_shift + x)
"""

import sys
import types

sys.path.insert(0, "/opt/trn_rl_repo")

import numpy as np
import ml_dtypes

# Problem shapes (hardcoded; kernel.py must be self-contained).
B = 4
D = 16
H = 64
WD = 64
C = 128
KS = 4
NCORES = 8
HSH = H // NCORES          # 8 H-rows per core
S = HSH * WD               # 512 spatial positions per (b, d) slice

# Set by test.py to run with NTFF profiling and stash exec time here.
PROFILE = False
NPROF = 4          # traced runs when PROFILE; min exec_time_ns is reported
LAST_EXEC_NS = None
LAST_EXEC_ALL = None
LAST_RESULT = None

_graph_cache = {}


def _install_ntff_hook():
    """Provide antenv.axon_hooks + register the NTFF profile hook if the
    image's antenv package lacks it (needed for trace=True under axon)."""
    try:
        from antenv import axon_hooks  # noqa: F401
        return
    except ImportError:
        pass
    try:
        import antenv
        from trn_agent_boot.trn_boot import _ntff_profile_via_ctypes
    except ImportError:
        return
    mod = types.ModuleType("antenv.axon_hooks")
    mod._hook = None

    def set_axon_ntff_profile_hook(h):
        mod._hook = h

    def get_axon_ntff_profile_hook():
        return mod._hook

    mod.set_axon_ntff_profile_hook = set_axon_ntff_profile_hook
    mod.get_axon_ntff_profile_hook = get_axon_ntff_profile_hook
    sys.modules["antenv.axon_hooks"] = mod
    antenv.axon_hooks = mod
    mod.set_axon_ntff_profile_hook(
        _ntff_profile_via_ctypes("/opt/axon/libaxon_pjrt.so")
    )


def _build_graph():
    import concourse.mybir as mybir
    from concourse import bacc
    from concourse.tile import TileContext

    nc = bacc.Bacc("TRN2", target_bir_lowering=False, debug=False,
                   num_devices=NCORES)
    xt = nc.declare_dram_parameter("xt", [C, B, D, S], mybir.dt.bfloat16,
                                   isOutput=False)
    # wk = [W0 | W1 | W2], Wk = diag(wk) W  (W2 with guarded w2)
    wk = nc.declare_dram_parameter("wk", [C, 3 * C], mybir.dt.bfloat16,
                                   isOutput=False)
    # misc columns: 0 = bias, 1 = r3 = w3/w2
    misc = nc.declare_dram_parameter("misc", [C, 2], mybir.dt.float32,
                                     isOutput=False)
    out = nc.declare_dram_parameter("out", [C, B, D, S], mybir.dt.bfloat16,
                                    isOutput=True)

    IDENT = mybir.ActivationFunctionType.Identity
    NU = D - 2                  # u_t defined for t in [0, NU); u_0 = x_0

    CHUNKS = {b: ([(0, 2), (2, 2), (4, 4), (8, 8)] if b == 0
                  else [(0, 8), (8, 8)]) for b in range(B)}

    with TileContext(nc) as tc:
        with (
            tc.tile_pool(name="consts", bufs=1) as cpool,
            tc.tile_pool(name="xin2", bufs=2) as xpool2,
            tc.tile_pool(name="xin4", bufs=1) as xpool4,
            tc.tile_pool(name="xin8", bufs=7) as xpool8,
            tc.tile_pool(name="sconv", bufs=2) as spool,
            tc.tile_pool(name="uconv", bufs=2) as upool,
            tc.tile_pool(name="ostage", bufs=16) as opool,
            tc.tile_pool(name="ps", bufs=4, space="PSUM") as pspool,
        ):
            # Const loads ride the ACT HWDGE ring; the SP ring carries the
            # x loads FIRST, then the output stores.  Single-ring FIFO =
            # store descriptors drain only after every load descriptor:
            # pure-read phase at ~358 GB/s, then pure-write — avoids the
            # ~4% HBM read/write-mix penalty measured when both streams
            # run concurrently.
            wk_sb = cpool.tile([C, 3 * C], mybir.dt.bfloat16)
            nc.scalar.dma_start(out=wk_sb[:], in_=wk[:])
            m_sb = cpool.tile([C, 2], mybir.dt.float32)
            nc.scalar.dma_start(out=m_sb[:], in_=misc[:])
            b_sb = m_sb[:, 0:1]
            r3_sb = m_sb[:, 1:2]

            # Keep the PE busy while the first x chunks stream in, so the
            # HAM clock gate flips to 2.4 GHz before the real matmuls start.
            warm_src = cpool.tile([C, S], mybir.dt.bfloat16)
            nc.vector.memset(warm_src[:], 0.0)
            for i in range(8):
                wt = pspool.tile([C, 2 * S], mybir.dt.float32, tag="ps",
                                 name=f"warm_{i}")
                nc.tensor.matmul(wt[:, 0:S], warm_src[:, 0:C], warm_src[:],
                                 start=True, stop=True)

            # SBUF holds the whole input (64 KB/partition), so every load
            # is issued up front and streams back-to-back.  Exact-size
            # pools keep the footprint at 64 KB (a shared-tag pool would
            # pad b0's small chunks to 8 KB slots).
            xpools = {2: xpool2, 4: xpool4, 8: xpool8}
            xh_all = {}
            for b in range(B):
                xh_all[b] = []
                for st, ln in CHUNKS[b]:
                    t = xpools[ln].tile([C, ln * S], mybir.dt.bfloat16,
                                        tag=f"xh{ln}", name=f"xh_{b}_{st}")
                    nc.sync.dma_start(
                        out=t[:],
                        in_=xt[:, b, st:st + ln].rearrange("c d s -> c (d s)"),
                    )
                    xh_all[b].append((st, ln, t))

            for b in range(B):
                load_chunks = CHUNKS[b]
                xh = xh_all[b]

                def x_span(t0, t1):
                    """AP covering x_{t0..t1-1}; must lie in one chunk."""
                    for st, ln, t in xh:
                        if st <= t0 and t1 <= st + ln:
                            return t[:, (t0 - st) * S:(t1 - st) * S]
                    raise AssertionError((t0, t1))

                # s_t = r3*x_t   for t in [0, NU-1)   (per load chunk)
                # u_t = s_{t-1} + x_t  for t in [1, NU); slot t*S in ut.
                # (u_0 = x_0 is aliased straight from the x tile.)
                st_tile = spool.tile([C, (NU - 1) * S], mybir.dt.bfloat16,
                                     tag="st", name=f"st_{b}")
                ut = upool.tile([C, NU * S], mybir.dt.bfloat16, tag="ut",
                                name=f"ut_{b}")

                for st, ln in load_chunks:
                    t0, t1 = st, min(NU - 1, st + ln)
                    if t0 < t1:
                        nc.vector.tensor_scalar_mul(
                            st_tile[:, t0 * S:t1 * S], x_span(t0, t1), r3_sb)

                # split [1, NU) at chunk starts so each tensor_add reads
                # x_t from a fixed tile (s is monolithic per b)
                cuts = sorted({1, NU} | {st for st, _ in load_chunks
                                         if 1 <= st <= NU})
                for t0, t1 in zip(cuts[:-1], cuts[1:]):
                    nc.vector.tensor_add(
                        ut[:, t0 * S:t1 * S],
                        st_tile[:, (t0 - 1) * S:(t1 - 1) * S],
                        x_span(t0, t1))

                def u_ap(tu):
                    if tu == 0:
                        return x_span(0, 1)
                    return ut[:, tu * S:(tu + 1) * S]

                # one staging tile per 4 depths -> 0.52 MB stores: starts
                # earlier and shortens the last evac->store drain tail
                ostages = []
                for h in range(4):
                    ot = opool.tile([C, 4 * S], mybir.dt.bfloat16, tag="ot",
                                    name=f"ot_{b}_{h}")
                    ostages.append(ot)

                # blocks of 2 depth slices -> one [C, 2S] PSUM tile
                # (2 banks; one matmul may only write one bank, so each
                # depth gets its own FD=512 matmuls).  Order W0,W0,W1,W1,
                # W2,W2 so the stationary reloads 3x per block, hidden
                # under the FD=512 matmul stream.
                for j in range(D // 2):
                    ps = pspool.tile([C, 2 * S], mybir.dt.float32, tag="ps",
                                     name=f"ps_{b}_{j}")

                    def psl(i):
                        return ps[:, i * S:(i + 1) * S]

                    dpair = (2 * j, 2 * j + 1)
                    for i, d in enumerate(dpair):
                        nc.tensor.matmul(psl(i), wk_sb[:, 0:C],
                                         x_span(d, d + 1),
                                         start=True, stop=(d == 0))
                    for i, d in enumerate(dpair):
                        if d >= 1:
                            nc.tensor.matmul(psl(i), wk_sb[:, C:2 * C],
                                             x_span(d - 1, d),
                                             start=False, stop=(d == 1))
                    for i, d in enumerate(dpair):
                        if d >= 2:
                            nc.tensor.matmul(psl(i), wk_sb[:, 2 * C:3 * C],
                                             u_ap(d - 2),
                                             start=False, stop=True)
                    h = j // 2
                    nc.scalar.activation(
                        out=ostages[h][:, (2 * j - 4 * h) * S:
                                       (2 * j - 4 * h + 2) * S],
                        in_=ps[:],
                        func=IDENT,
                        bias=b_sb,
                        scale=1.0,
                    )
                    if j % 2 == 1:
                        nc.sync.dma_start(
                            out=out[:, b, 4 * h:4 * h + 4].rearrange(
                                "c d s -> c (d s)"),
                            in_=ostages[h][:],
                        )
    nc.compile()
    return nc


def _get_graph():
    if "nc" not in _graph_cache:
        _graph_cache["nc"] = _build_graph()
    return _graph_cache["nc"]


def kernel(x, kernel, W, b):
    global LAST_EXEC_NS, LAST_RESULT
    from concourse.bass_utils import run_bass_kernel_spmd

    nc = _get_graph()

    x = np.asarray(x, np.float32)
    kernel = np.asarray(kernel, np.float32)
    W = np.asarray(W, np.float32)
    b = np.asarray(b, np.float32)

    # Host precompute: tap weights folded into the Linear; taps 2,3 share
    # W2 via the ratio r3 (exact: w2*(w3/w2) = w3).
    w_full = np.tile(kernel, (C // kernel.shape[0], 1))          # [C, KS]
    w0, w1, w2, w3 = (w_full[:, k] for k in range(KS))
    eps = np.float32(1e-12)
    g2 = np.where(np.abs(w2) < eps, np.copysign(eps, w2), w2)
    wk_cat = np.concatenate(
        [w0[:, None] * W, w1[:, None] * W, g2[:, None] * W], axis=1  # [C,3C]
    ).astype(ml_dtypes.bfloat16)
    misc = np.stack([b, w3 / g2], axis=1).astype(np.float32)     # [C, 2]

    # Channel-major transpose + H-shard + bf16.
    xbf = x.astype(ml_dtypes.bfloat16)
    xtr = np.transpose(xbf, (4, 0, 1, 2, 3))                     # [C,B,D,H,W]
    in_maps = []
    for i in range(NCORES):
        shard = np.ascontiguousarray(
            xtr[:, :, :, i * HSH:(i + 1) * HSH, :]
        ).reshape(C, B, D, S)
        in_maps.append({"xt": shard, "wk": wk_cat, "misc": misc})

    global LAST_EXEC_ALL
    core_ids = list(range(NCORES))
    res = None
    if PROFILE:
        _install_ntff_hook()
        try:
            # Warm run first: the NEFF compile on a cold cache must not
            # happen inside the NTFF capture window.
            run_bass_kernel_spmd(nc, in_maps, core_ids=core_ids)
            times = []
            for _ in range(max(1, NPROF)):
                res = run_bass_kernel_spmd(nc, in_maps, core_ids=core_ids,
                                           trace=True)
                times.append(res.exec_time_ns)
            LAST_EXEC_ALL = times
        except Exception as e:
            print(f"profile run failed ({type(e).__name__}: {e}); "
                  "falling back to non-traced run", file=sys.stderr)
            res = None
    if res is None:
        res = run_bass_kernel_spmd(nc, in_maps, core_ids=core_ids)
        LAST_EXEC_NS = res.exec_time_ns
    else:
        LAST_EXEC_NS = min(t for t in LAST_EXEC_ALL if t is not None)
    LAST_RESULT = res

    # Gather: shard_i[o, b, d, h*WD + w] -> full[b, d, i*HSH + h, w, o]
    o = np.stack([np.asarray(res.results[i]["out"]) for i in range(NCORES)],
                 axis=0).astype(np.float32)
    o = o.reshape(NCORES, C, B, D, HSH, WD)
    o = np.transpose(o, (2, 3, 0, 4, 5, 1)).reshape(B, D, H, WD, C)
    return np.ascontiguousarray(o)


# revision 14
# speedup vs baseline: 1.5362x; 1.0389x over previous
"""Trainium2 Bass kernel for CepstralBlock: causal depthwise conv along D
(K=4, per-channel weights) followed by a 128x128 Linear.

Math trick (v3): taps 0,1 stay as PSUM-accumulated matmuls; taps 2,3 are
folded PAIRWISE into the moving operand.  With Wk = diag(wk) W:

    out_d = W0^T x_d + W1^T x_{d-1} + W2^T (x_{d-2} + r3*x_{d-3})
          = W0^T x_d + W1^T x_{d-1} + W2^T u_{d-2}
    r3 = w3/w2   (per-channel, host-computed, guarded denominator)

u costs two cheap DVE passes (tensor_scalar_mul runs in 4x mode,
tensor_add in 2x mode -- the fused scalar_tensor_tensor op only has a 1x
uop and measured 2x slower than this pair).  The PE does 3 matmuls per
depth slice instead of 4 (measured PE-bound at 4).  Exact algebra:
W2^T r3 u-part = w2*(w3/w2) x W = w3 x W.  When |w2| ~ 0 the guarded
denominator introduces error <= 1e-12*|xW| -- negligible.

PSUM evacuation (+bias, f32->bf16) runs on the otherwise-idle Scalar
engine (activation Identity with per-partition bias), freeing the DVE.

Layout: host pre-transposes x to channel-major [C, B, D, S]; C=128 on
the SBUF partition axis (matmul contraction), no on-device transposes.
Sharding: data-parallel over H (64 -> 8 per core), 8 cores, no
collectives.  bf16 in/out, f32 PSUM accumulate.

Engine budget per core (target ~48us DMA-bound work window):
  DMA  ~47.6us (8.4 MB in + 8.4 MB out @ ~358 GB/s)  <- wall
  PE   ~41us (180 matmuls FD=512 + cold-clock ramp)
  ACT  ~36us (PSUM evac + bias)
  DVE  ~22us (s = r3*x, u = s_shift + x)
"""

import sys
import types

sys.path.insert(0, "/opt/trn_rl_repo")

import numpy as np
import ml_dtypes

# Problem shapes (hardcoded; kernel.py must be self-contained).
B = 4
D = 16
H = 64
WD = 64
C = 128
KS = 4
NCORES = 8
HSH = H // NCORES          # 8 H-rows per core
S = HSH * WD               # 512 spatial positions per (b, d) slice

# Set by test.py to run with NTFF profiling and stash exec time here.
PROFILE = False
NPROF = 4          # traced runs when PROFILE; min exec_time_ns is reported
LAST_EXEC_NS = None
LAST_EXEC_ALL = None
LAST_RESULT = None

_graph_cache = {}


def _install_ntff_hook():
    """Provide antenv.axon_hooks + register the NTFF profile hook if the
    image's antenv package lacks it (needed for trace=True under axon)."""
    try:
        from antenv import axon_hooks  # noqa: F401
        return
    except ImportError:
        pass
    try:
        import antenv
        from trn_agent_boot.trn_boot import _ntff_profile_via_ctypes
    except ImportError:
        return
    mod = types.ModuleType("antenv.axon_hooks")
    mod._hook = None

    def set_axon_ntff_profile_hook(h):
        mod._hook = h

    def get_axon_ntff_profile_hook():
        return mod._hook

    mod.set_axon_ntff_profile_hook = set_axon_ntff_profile_hook
    mod.get_axon_ntff_profile_hook = get_axon_ntff_profile_hook
    sys.modules["antenv.axon_hooks"] = mod
    antenv.axon_hooks = mod
    mod.set_axon_ntff_profile_hook(
        _ntff_profile_via_ctypes("/opt/axon/libaxon_pjrt.so")
    )


def _build_graph():
    import concourse.mybir as mybir
    from concourse import bacc
    from concourse.tile import TileContext

    nc = bacc.Bacc("TRN2", target_bir_lowering=False, debug=False,
                   num_devices=NCORES)
    xt = nc.declare_dram_parameter("xt", [C, B, D, S], mybir.dt.bfloat16,
                                   isOutput=False)
    # wk = [W0 | W1 | W2], Wk = diag(wk) W  (W0/W2 with guarded w0/w2)
    wk = nc.declare_dram_parameter("wk", [C, 3 * C], mybir.dt.bfloat16,
                                   isOutput=False)
    # misc columns: 0 = bias, 1 = r3 = w3/w2, 2 = r1 = w1/w0
    misc = nc.declare_dram_parameter("misc", [C, 3], mybir.dt.float32,
                                     isOutput=False)
    out = nc.declare_dram_parameter("out", [C, B, D, S], mybir.dt.bfloat16,
                                    isOutput=True)

    IDENT = mybir.ActivationFunctionType.Identity
    NU = D - 2                  # u_t defined for t in [0, NU); u_0 = x_0

    CHUNKS = {b: ([(0, 2), (2, 2), (4, 4), (8, 8)] if b == 0
                  else [(0, 8), (8, 8)]) for b in range(B)}

    with TileContext(nc) as tc:
        with (
            tc.tile_pool(name="consts", bufs=1) as cpool,
            tc.tile_pool(name="xin2", bufs=2) as xpool2,
            tc.tile_pool(name="xin4", bufs=1) as xpool4,
            tc.tile_pool(name="xin8", bufs=7) as xpool8,
            tc.tile_pool(name="sconv", bufs=2) as spool,
            tc.tile_pool(name="uconv", bufs=2) as upool,
            tc.tile_pool(name="s1conv", bufs=1) as s1pool,
            tc.tile_pool(name="vconv", bufs=1) as vpool,
            tc.tile_pool(name="ostage", bufs=10) as opool,
            tc.tile_pool(name="ps", bufs=4, space="PSUM") as pspool,
        ):
            # Const loads ride the ACT HWDGE ring; the SP ring carries the
            # x loads FIRST, then the output stores.  Single-ring FIFO =
            # store descriptors drain only after every load descriptor:
            # pure-read phase at ~358 GB/s, then pure-write — avoids the
            # ~4% HBM read/write-mix penalty measured when both streams
            # run concurrently.
            wk_sb = cpool.tile([C, 3 * C], mybir.dt.bfloat16)
            nc.scalar.dma_start(out=wk_sb[:], in_=wk[:])
            m_sb = cpool.tile([C, 3], mybir.dt.float32)
            nc.scalar.dma_start(out=m_sb[:], in_=misc[:])
            b_sb = m_sb[:, 0:1]
            r3_sb = m_sb[:, 1:2]
            r1_sb = m_sb[:, 2:3]

            # Keep the PE busy while the first x chunks stream in, so the
            # HAM clock gate flips to 2.4 GHz before the real matmuls start.
            warm_src = cpool.tile([C, S], mybir.dt.bfloat16)
            nc.vector.memset(warm_src[:], 0.0)
            for i in range(8):
                wt = pspool.tile([C, 2 * S], mybir.dt.float32, tag="ps",
                                 name=f"warm_{i}")
                nc.tensor.matmul(wt[:, 0:S], warm_src[:, 0:C], warm_src[:],
                                 start=True, stop=True)

            # SBUF holds the whole input (64 KB/partition), so every load
            # is issued up front and streams back-to-back.  Exact-size
            # pools keep the footprint at 64 KB (a shared-tag pool would
            # pad b0's small chunks to 8 KB slots).
            xpools = {2: xpool2, 4: xpool4, 8: xpool8}
            xh_all = {}
            for b in range(B):
                xh_all[b] = []
                for st, ln in CHUNKS[b]:
                    t = xpools[ln].tile([C, ln * S], mybir.dt.bfloat16,
                                        tag=f"xh{ln}", name=f"xh_{b}_{st}")
                    nc.sync.dma_start(
                        out=t[:],
                        in_=xt[:, b, st:st + ln].rearrange("c d s -> c (d s)"),
                    )
                    xh_all[b].append((st, ln, t))

            for b in range(B):
                load_chunks = CHUNKS[b]
                xh = xh_all[b]

                def x_span(t0, t1):
                    """AP covering x_{t0..t1-1}; must lie in one chunk."""
                    for st, ln, t in xh:
                        if st <= t0 and t1 <= st + ln:
                            return t[:, (t0 - st) * S:(t1 - st) * S]
                    raise AssertionError((t0, t1))

                # s_t = r3*x_t   for t in [0, NU-1)   (per load chunk)
                # u_t = s_{t-1} + x_t  for t in [1, NU); slot t*S in ut.
                # (u_0 = x_0 is aliased straight from the x tile.)
                st_tile = spool.tile([C, (NU - 1) * S], mybir.dt.bfloat16,
                                     tag="st", name=f"st_{b}")
                ut = upool.tile([C, NU * S], mybir.dt.bfloat16, tag="ut",
                                name=f"ut_{b}")

                for st, ln in load_chunks:
                    t0, t1 = st, min(NU - 1, st + ln)
                    if t0 < t1:
                        nc.vector.tensor_scalar_mul(
                            st_tile[:, t0 * S:t1 * S], x_span(t0, t1), r3_sb)

                # split [1, NU) at chunk starts so each tensor_add reads
                # x_t from a fixed tile (s is monolithic per b)
                cuts = sorted({1, NU} | {st for st, _ in load_chunks
                                         if 1 <= st <= NU})
                for t0, t1 in zip(cuts[:-1], cuts[1:]):
                    nc.vector.tensor_add(
                        ut[:, t0 * S:t1 * S],
                        st_tile[:, (t0 - 1) * S:(t1 - 1) * S],
                        x_span(t0, t1))

                def u_ap(tu):
                    if tu == 0:
                        return x_span(0, 1)
                    return ut[:, tu * S:(tu + 1) * S]

                # For the LAST batch, also fold taps 0,1 pairwise:
                # v_t = x_t + r1*x_{t-1}, out_d = W0^T v_d + W2^T u_{d-2}
                # -> 2 matmuls per depth.  PE is the tail-gating chain
                # after the loads drain; this shortens it by ~3 us at the
                # cost of two more DVE passes that fit in DVE's idle tail.
                fold01 = (b == B - 1)
                if fold01:
                    s1_tile = s1pool.tile([C, (D - 1) * S],
                                          mybir.dt.bfloat16, name="s1")
                    v_tile = vpool.tile([C, D * S], mybir.dt.bfloat16,
                                        name="vt")
                    for st, ln in load_chunks:
                        t0, t1 = st, min(D - 1, st + ln)
                        if t0 < t1:
                            nc.vector.tensor_scalar_mul(
                                s1_tile[:, t0 * S:t1 * S],
                                x_span(t0, t1), r1_sb)
                    vcuts = sorted({1, D} | {st for st, _ in load_chunks
                                             if 1 <= st <= D})
                    for t0, t1 in zip(vcuts[:-1], vcuts[1:]):
                        nc.vector.tensor_add(
                            v_tile[:, t0 * S:t1 * S],
                            s1_tile[:, (t0 - 1) * S:(t1 - 1) * S],
                            x_span(t0, t1))

                def v_ap(d):
                    if d == 0:
                        return x_span(0, 1)
                    return v_tile[:, d * S:(d + 1) * S]

                # one staging tile per 4 depths -> 0.52 MB stores: starts
                # earlier and shortens the last evac->store drain tail
                ostages = []
                for h in range(4):
                    ot = opool.tile([C, 4 * S], mybir.dt.bfloat16, tag="ot",
                                    name=f"ot_{b}_{h}")
                    ostages.append(ot)

                # blocks of 2 depth slices -> one [C, 2S] PSUM tile
                # (2 banks; one matmul may only write one bank, so each
                # depth gets its own FD=512 matmuls).  Stationary reloads
                # are grouped per weight and hidden under the FD=512
                # matmul stream.
                for j in range(D // 2):
                    ps = pspool.tile([C, 2 * S], mybir.dt.float32, tag="ps",
                                     name=f"ps_{b}_{j}")

                    def psl(i):
                        return ps[:, i * S:(i + 1) * S]

                    dpair = (2 * j, 2 * j + 1)
                    if fold01:
                        for i, d in enumerate(dpair):
                            nc.tensor.matmul(psl(i), wk_sb[:, 0:C],
                                             v_ap(d),
                                             start=True, stop=(d < 2))
                    else:
                        for i, d in enumerate(dpair):
                            nc.tensor.matmul(psl(i), wk_sb[:, 0:C],
                                             x_span(d, d + 1),
                                             start=True, stop=(d == 0))
                        for i, d in enumerate(dpair):
                            if d >= 1:
                                nc.tensor.matmul(psl(i), wk_sb[:, C:2 * C],
                                                 x_span(d - 1, d),
                                                 start=False, stop=(d == 1))
                    for i, d in enumerate(dpair):
                        if d >= 2:
                            nc.tensor.matmul(psl(i), wk_sb[:, 2 * C:3 * C],
                                             u_ap(d - 2),
                                             start=False, stop=True)
                    h = j // 2
                    nc.scalar.activation(
                        out=ostages[h][:, (2 * j - 4 * h) * S:
                                       (2 * j - 4 * h + 2) * S],
                        in_=ps[:],
                        func=IDENT,
                        bias=b_sb,
                        scale=1.0,
                    )
                    if j % 2 == 1:
                        nc.sync.dma_start(
                            out=out[:, b, 4 * h:4 * h + 4].rearrange(
                                "c d s -> c (d s)"),
                            in_=ostages[h][:],
                        )
    nc.compile()
    return nc


def _get_graph():
    if "nc" not in _graph_cache:
        _graph_cache["nc"] = _build_graph()
    return _graph_cache["nc"]


def kernel(x, kernel, W, b):
    global LAST_EXEC_NS, LAST_RESULT
    from concourse.bass_utils import run_bass_kernel_spmd

    nc = _get_graph()

    x = np.asarray(x, np.float32)
    kernel = np.asarray(kernel, np.float32)
    W = np.asarray(W, np.float32)
    b = np.asarray(b, np.float32)

    # Host precompute: tap weights folded into the Linear; taps 2,3 share
    # W2 via the ratio r3 (exact: w2*(w3/w2) = w3).
    w_full = np.tile(kernel, (C // kernel.shape[0], 1))          # [C, KS]
    w0, w1, w2, w3 = (w_full[:, k] for k in range(KS))
    eps = np.float32(1e-12)
    g0 = np.where(np.abs(w0) < eps, np.copysign(eps, w0), w0)
    g2 = np.where(np.abs(w2) < eps, np.copysign(eps, w2), w2)
    wk_cat = np.concatenate(
        [g0[:, None] * W, w1[:, None] * W, g2[:, None] * W], axis=1  # [C,3C]
    ).astype(ml_dtypes.bfloat16)
    misc = np.stack([b, w3 / g2, w1 / g0], axis=1).astype(np.float32)  # [C,3]

    # Channel-major transpose + H-shard + bf16.
    xbf = x.astype(ml_dtypes.bfloat16)
    xtr = np.transpose(xbf, (4, 0, 1, 2, 3))                     # [C,B,D,H,W]
    in_maps = []
    for i in range(NCORES):
        shard = np.ascontiguousarray(
            xtr[:, :, :, i * HSH:(i + 1) * HSH, :]
        ).reshape(C, B, D, S)
        in_maps.append({"xt": shard, "wk": wk_cat, "misc": misc})

    global LAST_EXEC_ALL
    core_ids = list(range(NCORES))
    res = None
    if PROFILE:
        _install_ntff_hook()
        try:
            # Warm run first: the NEFF compile on a cold cache must not
            # happen inside the NTFF capture window.
            run_bass_kernel_spmd(nc, in_maps, core_ids=core_ids)
            times = []
            for _ in range(max(1, NPROF)):
                res = run_bass_kernel_spmd(nc, in_maps, core_ids=core_ids,
                                           trace=True)
                times.append(res.exec_time_ns)
            LAST_EXEC_ALL = times
        except Exception as e:
            print(f"profile run failed ({type(e).__name__}: {e}); "
                  "falling back to non-traced run", file=sys.stderr)
            res = None
    if res is None:
        res = run_bass_kernel_spmd(nc, in_maps, core_ids=core_ids)
        LAST_EXEC_NS = res.exec_time_ns
    else:
        LAST_EXEC_NS = min(t for t in LAST_EXEC_ALL if t is not None)
    LAST_RESULT = res

    # Gather: shard_i[o, b, d, h*WD + w] -> full[b, d, i*HSH + h, w, o]
    o = np.stack([np.asarray(res.results[i]["out"]) for i in range(NCORES)],
                 axis=0).astype(np.float32)
    o = o.reshape(NCORES, C, B, D, HSH, WD)
    o = np.transpose(o, (2, 3, 0, 4, 5, 1)).reshape(B, D, H, WD, C)
    return np.ascontiguousarray(o)


# revision 16
# speedup vs baseline: 1.5684x; 1.0209x over previous
"""Trainium2 Bass kernel for CepstralBlock: causal depthwise conv along D
(K=4, per-channel weights) followed by a 128x128 Linear.

Math trick (v3): taps 0,1 stay as PSUM-accumulated matmuls; taps 2,3 are
folded PAIRWISE into the moving operand.  With Wk = diag(wk) W:

    out_d = W0^T x_d + W1^T x_{d-1} + W2^T (x_{d-2} + r3*x_{d-3})
          = W0^T x_d + W1^T x_{d-1} + W2^T u_{d-2}
    r3 = w3/w2   (per-channel, host-computed, guarded denominator)

u costs two cheap DVE passes (tensor_scalar_mul runs in 4x mode,
tensor_add in 2x mode -- the fused scalar_tensor_tensor op only has a 1x
uop and measured 2x slower than this pair).  The PE does 3 matmuls per
depth slice instead of 4 (measured PE-bound at 4).  Exact algebra:
W2^T r3 u-part = w2*(w3/w2) x W = w3 x W.  When |w2| ~ 0 the guarded
denominator introduces error <= 1e-12*|xW| -- negligible.

PSUM evacuation (+bias, f32->bf16) runs on the otherwise-idle Scalar
engine (activation Identity with per-partition bias), freeing the DVE.

Layout: host pre-transposes x to channel-major [C, B, D, S]; C=128 on
the SBUF partition axis (matmul contraction), no on-device transposes.
Sharding: data-parallel over H (64 -> 8 per core), 8 cores, no
collectives.  bf16 in/out, f32 PSUM accumulate.

Engine budget per core (target ~48us DMA-bound work window):
  DMA  ~47.6us (8.4 MB in + 8.4 MB out @ ~358 GB/s)  <- wall
  PE   ~41us (180 matmuls FD=512 + cold-clock ramp)
  ACT  ~36us (PSUM evac + bias)
  DVE  ~22us (s = r3*x, u = s_shift + x)
"""

import sys
import types

sys.path.insert(0, "/opt/trn_rl_repo")

import numpy as np
import ml_dtypes

# Problem shapes (hardcoded; kernel.py must be self-contained).
B = 4
D = 16
H = 64
WD = 64
C = 128
KS = 4
NCORES = 8
HSH = H // NCORES          # 8 H-rows per core
S = HSH * WD               # 512 spatial positions per (b, d) slice

# Set by test.py to run with NTFF profiling and stash exec time here.
PROFILE = False
NPROF = 4          # traced runs when PROFILE; min exec_time_ns is reported
LAST_EXEC_NS = None
LAST_EXEC_ALL = None
LAST_RESULT = None

_graph_cache = {}


def _install_ntff_hook():
    """Provide antenv.axon_hooks + register the NTFF profile hook if the
    image's antenv package lacks it (needed for trace=True under axon)."""
    try:
        from antenv import axon_hooks  # noqa: F401
        return
    except ImportError:
        pass
    try:
        import antenv
        from trn_agent_boot.trn_boot import _ntff_profile_via_ctypes
    except ImportError:
        return
    mod = types.ModuleType("antenv.axon_hooks")
    mod._hook = None

    def set_axon_ntff_profile_hook(h):
        mod._hook = h

    def get_axon_ntff_profile_hook():
        return mod._hook

    mod.set_axon_ntff_profile_hook = set_axon_ntff_profile_hook
    mod.get_axon_ntff_profile_hook = get_axon_ntff_profile_hook
    sys.modules["antenv.axon_hooks"] = mod
    antenv.axon_hooks = mod
    mod.set_axon_ntff_profile_hook(
        _ntff_profile_via_ctypes("/opt/axon/libaxon_pjrt.so")
    )


def _build_graph():
    import concourse.mybir as mybir
    from concourse import bacc
    from concourse.tile import TileContext

    nc = bacc.Bacc("TRN2", target_bir_lowering=False, debug=False,
                   num_devices=NCORES)
    xt = nc.declare_dram_parameter("xt", [C, B, D, S], mybir.dt.bfloat16,
                                   isOutput=False)
    # wk = [W0 | W1 | W2], Wk = diag(wk) W  (W0/W2 with guarded w0/w2)
    wk = nc.declare_dram_parameter("wk", [C, 3 * C], mybir.dt.bfloat16,
                                   isOutput=False)
    # misc columns: 0 = bias, 1 = r3 = w3/w2, 2 = r1 = w1/w0
    misc = nc.declare_dram_parameter("misc", [C, 3], mybir.dt.float32,
                                     isOutput=False)
    out = nc.declare_dram_parameter("out", [C, B, D, S], mybir.dt.bfloat16,
                                    isOutput=True)

    IDENT = mybir.ActivationFunctionType.Identity
    NU = D - 2                  # u_t defined for t in [0, NU); u_0 = x_0

    CHUNKS = {b: ([(0, 2), (2, 2), (4, 4), (8, 8)] if b == 0
                  else [(0, 8), (8, 8)]) for b in range(B)}

    with TileContext(nc) as tc:
        with (
            tc.tile_pool(name="consts", bufs=1) as cpool,
            tc.tile_pool(name="xin2", bufs=2) as xpool2,
            tc.tile_pool(name="xin4", bufs=1) as xpool4,
            tc.tile_pool(name="xin8", bufs=7) as xpool8,
            tc.tile_pool(name="sconv", bufs=2) as spool,
            tc.tile_pool(name="uconv", bufs=2) as upool,
            tc.tile_pool(name="s1conv", bufs=1) as s1pool,
            tc.tile_pool(name="vconv", bufs=1) as vpool,
            tc.tile_pool(name="ostage", bufs=10) as opool,
            tc.tile_pool(name="ps", bufs=4, space="PSUM") as pspool,
        ):
            # Const loads ride the ACT HWDGE ring; the SP ring carries the
            # x loads FIRST, then the output stores.  Single-ring FIFO =
            # store descriptors drain only after every load descriptor:
            # pure-read phase at ~358 GB/s, then pure-write — avoids the
            # ~4% HBM read/write-mix penalty measured when both streams
            # run concurrently.
            wk_sb = cpool.tile([C, 3 * C], mybir.dt.bfloat16)
            nc.scalar.dma_start(out=wk_sb[:], in_=wk[:])
            m_sb = cpool.tile([C, 3], mybir.dt.float32)
            nc.scalar.dma_start(out=m_sb[:], in_=misc[:])
            b_sb = m_sb[:, 0:1]
            r3_sb = m_sb[:, 1:2]
            r1_sb = m_sb[:, 2:3]

            # Keep the PE busy while the first x chunks stream in, so the
            # HAM clock gate flips to 2.4 GHz before the real matmuls start.
            warm_src = cpool.tile([C, S], mybir.dt.bfloat16)
            nc.vector.memset(warm_src[:], 0.0)
            for i in range(16):
                wt = pspool.tile([C, 2 * S], mybir.dt.float32, tag="ps",
                                 name=f"warm_{i}")
                nc.tensor.matmul(wt[:, 0:S], warm_src[:, 0:C], warm_src[:],
                                 start=True, stop=True)

            # SBUF holds the whole input (64 KB/partition), so every load
            # is issued up front and streams back-to-back.  Exact-size
            # pools keep the footprint at 64 KB (a shared-tag pool would
            # pad b0's small chunks to 8 KB slots).
            xpools = {2: xpool2, 4: xpool4, 8: xpool8}
            xh_all = {}
            for b in range(B):
                xh_all[b] = []
                for st, ln in CHUNKS[b]:
                    t = xpools[ln].tile([C, ln * S], mybir.dt.bfloat16,
                                        tag=f"xh{ln}", name=f"xh_{b}_{st}")
                    nc.sync.dma_start(
                        out=t[:],
                        in_=xt[:, b, st:st + ln].rearrange("c d s -> c (d s)"),
                    )
                    xh_all[b].append((st, ln, t))

            for b in range(B):
                load_chunks = CHUNKS[b]
                xh = xh_all[b]

                def x_span(t0, t1):
                    """AP covering x_{t0..t1-1}; must lie in one chunk."""
                    for st, ln, t in xh:
                        if st <= t0 and t1 <= st + ln:
                            return t[:, (t0 - st) * S:(t1 - st) * S]
                    raise AssertionError((t0, t1))

                # s_t = r3*x_t   for t in [0, NU-1)   (per load chunk)
                # u_t = s_{t-1} + x_t  for t in [1, NU); slot t*S in ut.
                # (u_0 = x_0 is aliased straight from the x tile.)
                st_tile = spool.tile([C, (NU - 1) * S], mybir.dt.bfloat16,
                                     tag="st", name=f"st_{b}")
                ut = upool.tile([C, NU * S], mybir.dt.bfloat16, tag="ut",
                                name=f"ut_{b}")

                for st, ln in load_chunks:
                    t0, t1 = st, min(NU - 1, st + ln)
                    if t0 < t1:
                        nc.vector.tensor_scalar_mul(
                            st_tile[:, t0 * S:t1 * S], x_span(t0, t1), r3_sb)

                # split [1, NU) at chunk starts so each tensor_add reads
                # x_t from a fixed tile (s is monolithic per b)
                cuts = sorted({1, NU} | {st for st, _ in load_chunks
                                         if 1 <= st <= NU})
                for t0, t1 in zip(cuts[:-1], cuts[1:]):
                    nc.vector.tensor_add(
                        ut[:, t0 * S:t1 * S],
                        st_tile[:, (t0 - 1) * S:(t1 - 1) * S],
                        x_span(t0, t1))

                def u_ap(tu):
                    if tu == 0:
                        return x_span(0, 1)
                    return ut[:, tu * S:(tu + 1) * S]

                # For the LAST batch, also fold taps 0,1 pairwise:
                # v_t = x_t + r1*x_{t-1}, out_d = W0^T v_d + W2^T u_{d-2}
                # -> 2 matmuls per depth.  PE is the tail-gating chain
                # after the loads drain; this shortens it by ~3 us at the
                # cost of two more DVE passes that fit in DVE's idle tail.
                fold01 = (b == B - 1)
                if fold01:
                    s1_tile = s1pool.tile([C, (D - 1) * S],
                                          mybir.dt.bfloat16, name="s1")
                    v_tile = vpool.tile([C, D * S], mybir.dt.bfloat16,
                                        name="vt")
                    for st, ln in load_chunks:
                        t0, t1 = st, min(D - 1, st + ln)
                        if t0 < t1:
                            nc.vector.tensor_scalar_mul(
                                s1_tile[:, t0 * S:t1 * S],
                                x_span(t0, t1), r1_sb)
                    vcuts = sorted({1, D} | {st for st, _ in load_chunks
                                             if 1 <= st <= D})
                    for t0, t1 in zip(vcuts[:-1], vcuts[1:]):
                        nc.vector.tensor_add(
                            v_tile[:, t0 * S:t1 * S],
                            s1_tile[:, (t0 - 1) * S:(t1 - 1) * S],
                            x_span(t0, t1))

                def v_ap(d):
                    if d == 0:
                        return x_span(0, 1)
                    return v_tile[:, d * S:(d + 1) * S]

                # one staging tile per 4 depths -> 0.52 MB stores: starts
                # earlier and shortens the last evac->store drain tail
                ostages = []
                for h in range(4):
                    ot = opool.tile([C, 4 * S], mybir.dt.bfloat16, tag="ot",
                                    name=f"ot_{b}_{h}")
                    ostages.append(ot)

                # blocks of 2 depth slices -> one [C, 2S] PSUM tile
                # (2 banks; one matmul may only write one bank, so each
                # depth gets its own FD=512 matmuls).  Stationary reloads
                # are grouped per weight and hidden under the FD=512
                # matmul stream.
                for j in range(D // 2):
                    ps = pspool.tile([C, 2 * S], mybir.dt.float32, tag="ps",
                                     name=f"ps_{b}_{j}")

                    def psl(i):
                        return ps[:, i * S:(i + 1) * S]

                    dpair = (2 * j, 2 * j + 1)
                    if fold01:
                        for i, d in enumerate(dpair):
                            nc.tensor.matmul(psl(i), wk_sb[:, 0:C],
                                             v_ap(d),
                                             start=True, stop=(d < 2))
                    else:
                        for i, d in enumerate(dpair):
                            nc.tensor.matmul(psl(i), wk_sb[:, 0:C],
                                             x_span(d, d + 1),
                                             start=True, stop=(d == 0))
                        for i, d in enumerate(dpair):
                            if d >= 1:
                                nc.tensor.matmul(psl(i), wk_sb[:, C:2 * C],
                                                 x_span(d - 1, d),
                                                 start=False, stop=(d == 1))
                    for i, d in enumerate(dpair):
                        if d >= 2:
                            nc.tensor.matmul(psl(i), wk_sb[:, 2 * C:3 * C],
                                             u_ap(d - 2),
                                             start=False, stop=True)
                    h = j // 2
                    o_ap = ostages[h][:, (2 * j - 4 * h) * S:
                                      (2 * j - 4 * h + 2) * S]
                    if fold01 and j % 2 == 1:
                        # DVE is idle during the tail; splitting the last
                        # batch's evacs between both engines halves the
                        # final PSUM->SBUF drain chain.
                        nc.vector.tensor_scalar_add(o_ap, ps[:], b_sb)
                    else:
                        nc.scalar.activation(
                            out=o_ap,
                            in_=ps[:],
                            func=IDENT,
                            bias=b_sb,
                            scale=1.0,
                        )
                    if j % 2 == 1:
                        if fold01 and j == D // 2 - 1:
                            # final two stores at 0.26 MB: shorter last
                            # evac->store drain
                            for q in range(2):
                                nc.sync.dma_start(
                                    out=out[:, b, 4 * h + 2 * q:
                                            4 * h + 2 * q + 2].rearrange(
                                        "c d s -> c (d s)"),
                                    in_=ostages[h][:, 2 * q * S:
                                                   (2 * q + 2) * S],
                                )
                        else:
                            nc.sync.dma_start(
                                out=out[:, b, 4 * h:4 * h + 4].rearrange(
                                    "c d s -> c (d s)"),
                                in_=ostages[h][:],
                            )
    nc.compile()
    return nc


def _get_graph():
    if "nc" not in _graph_cache:
        _graph_cache["nc"] = _build_graph()
    return _graph_cache["nc"]


def kernel(x, kernel, W, b):
    global LAST_EXEC_NS, LAST_RESULT
    from concourse.bass_utils import run_bass_kernel_spmd

    nc = _get_graph()

    x = np.asarray(x, np.float32)
    kernel = np.asarray(kernel, np.float32)
    W = np.asarray(W, np.float32)
    b = np.asarray(b, np.float32)

    # Host precompute: tap weights folded into the Linear; taps 2,3 share
    # W2 via the ratio r3 (exact: w2*(w3/w2) = w3).
    w_full = np.tile(kernel, (C // kernel.shape[0], 1))          # [C, KS]
    w0, w1, w2, w3 = (w_full[:, k] for k in range(KS))
    eps = np.float32(1e-12)
    g0 = np.where(np.abs(w0) < eps, np.copysign(eps, w0), w0)
    g2 = np.where(np.abs(w2) < eps, np.copysign(eps, w2), w2)
    wk_cat = np.concatenate(
        [g0[:, None] * W, w1[:, None] * W, g2[:, None] * W], axis=1  # [C,3C]
    ).astype(ml_dtypes.bfloat16)
    misc = np.stack([b, w3 / g2, w1 / g0], axis=1).astype(np.float32)  # [C,3]

    # Channel-major transpose + H-shard + bf16.
    xbf = x.astype(ml_dtypes.bfloat16)
    xtr = np.transpose(xbf, (4, 0, 1, 2, 3))                     # [C,B,D,H,W]
    in_maps = []
    for i in range(NCORES):
        shard = np.ascontiguousarray(
            xtr[:, :, :, i * HSH:(i + 1) * HSH, :]
        ).reshape(C, B, D, S)
        in_maps.append({"xt": shard, "wk": wk_cat, "misc": misc})

    global LAST_EXEC_ALL
    core_ids = list(range(NCORES))
    res = None
    if PROFILE:
        _install_ntff_hook()
        try:
            # Warm run first: the NEFF compile on a cold cache must not
            # happen inside the NTFF capture window.
            run_bass_kernel_spmd(nc, in_maps, core_ids=core_ids)
            times = []
            for _ in range(max(1, NPROF)):
                res = run_bass_kernel_spmd(nc, in_maps, core_ids=core_ids,
                                           trace=True)
                times.append(res.exec_time_ns)
            LAST_EXEC_ALL = times
        except Exception as e:
            print(f"profile run failed ({type(e).__name__}: {e}); "
                  "falling back to non-traced run", file=sys.stderr)
            res = None
    if res is None:
        res = run_bass_kernel_spmd(nc, in_maps, core_ids=core_ids)
        LAST_EXEC_NS = res.exec_time_ns
    else:
        LAST_EXEC_NS = min(t for t in LAST_EXEC_ALL if t is not None)
    LAST_RESULT = res

    # Gather: shard_i[o, b, d, h*WD + w] -> full[b, d, i*HSH + h, w, o]
    o = np.stack([np.asarray(res.results[i]["out"]) for i in range(NCORES)],
                 axis=0).astype(np.float32)
    o = o.reshape(NCORES, C, B, D, HSH, WD)
    o = np.transpose(o, (2, 3, 0, 4, 5, 1)).reshape(B, D, H, WD, C)
    return np.ascontiguousarray(o)
